# revision 34
# baseline (speedup 1.0000x reference)
"""Causal self-attention (GQA + RoPE) Trainium2 Bass kernel, 8 NeuronCores.

Problem: B=2, T=2048, C=2048, n_head=16, n_kv_head=4, head_dim=128.

Sharding: 2-way batch DP x 4-way head TP. Core c = 4*b + g handles batch b,
kv head g, q heads [4g, 4g+4). wq/wk/wv column-sharded per head group, wo
row-sharded; per-core partial outputs are summed on the host.

Device dataflow (fp16 matmul operands, fp32 PSUM), fused per-quarter
pipeline  PROJ(q) -> WO(q-1) -> ATT(q)  so the PE never drains:

  PROJ(q):  Q^T/K^T/V^T [d, 512 t] accumulated over C in 16 chunks from a
            resident x-quarter tile; V re-transposed to [s, dv] via PE;
            RoPE applied in [d, t] layout (swap halves via SBUF->SBUF DMA).
  ATT(q):   S^T[s_blk, t] = K^T_blk.T @ Q^T per 128-s-block; causal mask on
            diagonal blocks added by an extra matmul with precomputed
            triangular factors (ramp trick, no DVE on the critical path);
            exp on ACT over a [128, <=1024] wide tile (2 s-blocks/op);
            denominator via ones-matmul; O^T accumulated per head;
            1/den via vector.reciprocal_approx_fast; O^T normalized on DVE.
  WO(q):    out^T partial [128 rows, 512 t] = sum_h woX.T @ O^T, drained
            alternately on ACT/DVE into an out tile, one big DMA per quarter.

PSUM plan (8 banks): tag A = 2 x [128,1024] (proj q-head pair accumulators /
wide score tiles), tag B = 2 x [128,512] (k/v accs, o accs, wo accs),
tag C = 2 x [128,512] (v-transpose tiles, denominator accs). Ring reuse of
tags encodes the pipeline dependencies.

All DRAM tensors are laid out host-side so every DMA moves >=4KB contiguous
per partition (the baseline's 1KB rows were descriptor-rate limited).
"""

import sys

sys.path.insert(0, "/opt/trn_rl_repo")

import numpy as np

import concourse.bass as bass
import concourse.mybir as mybir
import concourse.tile as tile
from concourse import bacc
from concourse.bass_utils import run_bass_kernel_spmd
from concourse.masks import make_identity

F32 = mybir.dt.float32
F16 = mybir.dt.float16
AF = mybir.ActivationFunctionType

B, T, C = 2, 2048, 2048
N_HEAD, N_KV_HEAD = 16, 4
HD = 128                 # head dim
QH = 4                   # q heads per core
TQ = 512                 # t-chunk
NQ = T // TQ             # 4 quarters
CK = C // 128            # 16 contraction chunks of 128
SCALE = 1.0 / float(np.sqrt(HD))
MASK_NEG = -30000.0

_CACHE = {}


def _build_nc():
    nc = bacc.Bacc("TRN2", target_bir_lowering=False, debug=False, num_devices=8)

    xQ = nc.dram_tensor("xQ", [128, NQ, CK, TQ], F16, kind="ExternalInput").ap()
    wqX = nc.dram_tensor("wqX", [128, CK, QH * HD], F16, kind="ExternalInput").ap()
    wkX = nc.dram_tensor("wkX", [128, CK, HD], F16, kind="ExternalInput").ap()
    wvX = nc.dram_tensor("wvX", [128, CK, HD], F16, kind="ExternalInput").ap()
    woX = nc.dram_tensor("woX", [128, CK, QH, HD], F16, kind="ExternalInput").ap()
    cosX = nc.dram_tensor("cosX", [HD, T], F16, kind="ExternalInput").ap()
    sinX = nc.dram_tensor("sinX", [HD, T], F16, kind="ExternalInput").ap()
    permX = nc.dram_tensor("permX", [128, 128], F16, kind="ExternalInput").ap()
    outX = nc.dram_tensor("outX", [128, NQ, CK, TQ], F16, kind="ExternalOutput").ap()

    with tile.TileContext(nc) as tc:
        _emit(nc, tc, xQ, wqX, wkX, wvX, woX, cosX, sinX, permX, outX)

    nc.compile()
    return nc


def _emit(nc, tc, xQ, wqX, wkX, wvX, woX, cosX, sinX, permX, outX):
    import contextlib

    ctx = contextlib.ExitStack()
    with ctx:
        singles = ctx.enter_context(tc.tile_pool(name="singles", bufs=1))
        psum = ctx.enter_context(tc.tile_pool(name="ps", bufs=1, space="PSUM"))
        xring = ctx.enter_context(tc.tile_pool(name="xr", bufs=2))
        ppool = ctx.enter_context(tc.tile_pool(name="pp", bufs=4))
        rpool = ctx.enter_context(tc.tile_pool(name="rp", bufs=4))
        vtpool = ctx.enter_context(tc.tile_pool(name="vtp", bufs=2))
        outsb = ctx.enter_context(tc.tile_pool(name="ou", bufs=2))

        # ---- resident weights / activations ----
        wq_sb = singles.tile([128, CK, QH * HD], F16)
        wk_sb = singles.tile([128, CK, HD], F16)
        wv_sb = singles.tile([128, CK, HD], F16)
        wo_sb = singles.tile([128, CK, QH, HD], F16)
        cos_sb = singles.tile([HD, T], F16)
        sin_sb = singles.tile([HD, T], F16)
        qT = [singles.tile([128, QH, TQ], F16, name=f"qT{q}") for q in range(NQ)]
        kT = [singles.tile([128, TQ], F16, name=f"kT{q}") for q in range(NQ)]
        vS = [singles.tile([128, 4, HD], F16, name=f"vS{q}") for q in range(NQ)]
        oT = [singles.tile([128, QH, TQ], F16, name=f"oT{q}") for q in range(NQ)]

        # ---- input DMAs, in startup-critical order ----
        # x quarters as half-tiles (chunks 0-7 / 8-15) so PROJ(0) can start
        # after ~2MB of traffic instead of ~5MB.
        perm_sb = singles.tile([128, 128], F16)
        xt = [None] * NQ

        def x_load(q, split=False):
            a = xring.tile([128, 2, TQ], F16, tag="xa", bufs=2, name=f"x{q}a")
            b = xring.tile([128, 6, TQ], F16, tag="xb", bufs=2, name=f"x{q}b")
            c = xring.tile([128, 8, TQ], F16, tag="xc", bufs=2, name=f"x{q}c")
            nc.sync.dma_start(out=a, in_=xQ[:, q, 0:2, :])
            if split:
                return (a, b, c)
            nc.sync.dma_start(out=b, in_=xQ[:, q, 2:8, :])
            nc.sync.dma_start(out=c, in_=xQ[:, q, 8:16, :])
            xt[q] = (a, b, c)

        def xk_of(q, k):
            a, b, c = xt[q]
            if k < 2:
                return a[:, k, :]
            if k < 8:
                return b[:, k - 2, :]
            return c[:, k - 8, :]

        nc.sync.dma_start(out=wq_sb[:, 0:1, :], in_=wqX[:, 0:1, :])
        nc.sync.dma_start(out=wk_sb, in_=wkX)
        nc.sync.dma_start(out=wv_sb, in_=wvX)
        abc = x_load(0, split=True)   # x0a issued now
        nc.sync.dma_start(out=wq_sb[:, 1:4, :], in_=wqX[:, 1:4, :])
        nc.sync.dma_start(out=abc[1], in_=xQ[:, 0, 2:8, :])
        nc.sync.dma_start(out=wq_sb[:, 4:8, :], in_=wqX[:, 4:8, :])
        nc.sync.dma_start(out=abc[2], in_=xQ[:, 0, 8:16, :])
        xt[0] = abc
        nc.sync.dma_start(out=wq_sb[:, 8:12, :], in_=wqX[:, 8:12, :])
        nc.sync.dma_start(out=wq_sb[:, 12:16, :], in_=wqX[:, 12:16, :])
        nc.sync.dma_start(out=cos_sb, in_=cosX)
        nc.sync.dma_start(out=sin_sb, in_=sinX)
        nc.sync.dma_start(out=perm_sb, in_=permX)
        x_load(1)
        nc.sync.dma_start(out=wo_sb, in_=woX)

        # ---- constants ----
        ident = singles.tile([128, 128], F16)
        make_identity(nc, ident)
        ones_sq = singles.tile([128, 128], F16)
        nc.vector.memset(ones_sq, 1.0)
        # causal ramp factors: mask[s,t'] = sum_r A[r,s]*Bm[r,t']
        #   = MASK_NEG * (s - t') for s > t', 0 otherwise  (t' = col in diag blk)
        maskA = singles.tile([128, 128], F16)
        nc.gpsimd.memset(maskA, 1.0)
        # keep 1 where s - r - 1 >= 0 (r < s), else 0
        nc.gpsimd.affine_select(
            out=maskA, in_=maskA, compare_op=mybir.AluOpType.is_ge,
            fill=0.0, base=-1, pattern=[[1, 128]], channel_multiplier=-1)
        maskB = singles.tile([128, 128], F16)
        nc.gpsimd.memset(maskB, MASK_NEG)
        # keep MASK_NEG where r - t' >= 0 (r >= t'), else 0
        nc.gpsimd.affine_select(
            out=maskB, in_=maskB, compare_op=mybir.AluOpType.is_ge,
            fill=0.0, base=0, pattern=[[-1, 128]], channel_multiplier=1)




        def proj(q):
            q01 = psum.tile([128, 2 * TQ], F32, tag="A", bufs=2, name=f"q01_{q}")
            q23 = psum.tile([128, 2 * TQ], F32, tag="A", bufs=2, name=f"q23_{q}")
            kacc = psum.tile([128, TQ], F32, tag="B", bufs=2, name=f"kacc{q}")
            vacc = psum.tile([128, TQ], F32, tag="B", bufs=2, name=f"vacc{q}")
            for k in range(CK):
                xk = xk_of(q, k)
                st, sp = (k == 0), (k == CK - 1)
                nc.tensor.matmul(q01[:, 0:TQ], wq_sb[:, k, 0:128], xk,
                                 start=st, stop=sp)
                nc.tensor.matmul(q01[:, TQ:2 * TQ], wq_sb[:, k, 128:256], xk,
                                 start=st, stop=sp)
                nc.tensor.matmul(q23[:, 0:TQ], wq_sb[:, k, 256:384], xk,
                                 start=st, stop=sp)
                nc.tensor.matmul(q23[:, TQ:2 * TQ], wq_sb[:, k, 384:512], xk,
                                 start=st, stop=sp)
                nc.tensor.matmul(kacc, wk_sb[:, k, :], xk, start=st, stop=sp)
                nc.tensor.matmul(vacc, wv_sb[:, k, :], xk, start=st, stop=sp)
            # Drains on ACT in RoPE-critical order: q0 and k first.
            nc.scalar.copy(out=qT[q][:, 0, :], in_=q01[:, 0:TQ])
            nc.scalar.copy(out=kT[q], in_=kacc)
            nc.scalar.copy(out=qT[q][:, 1, :], in_=q01[:, TQ:2 * TQ])
            nc.scalar.copy(out=qT[q][:, 2, :], in_=q23[:, 0:TQ])
            nc.scalar.copy(out=qT[q][:, 3, :], in_=q23[:, TQ:2 * TQ])
            vt = vtpool.tile([128, TQ], F16, tag="v")
            nc.vector.tensor_copy(out=vt, in_=vacc)
            # RoPE in [d, t] layout: rows 0..63 even dims, 64..127 odd dims.
            # Half-swap via PE permutation matmul (fast; SBUF->SBUF DMA is
            # too slow for the critical path), then mul/mul/add on DVE.
            cs = cos_sb[:, TQ * q:TQ * (q + 1)]
            sn = sin_sb[:, TQ * q:TQ * (q + 1)]
            tgts = [qT[q][:, 0, :], kT[q]] + [qT[q][:, h, :] for h in (1, 2, 3)]
            sw_ps = []
            for ti, tgt in enumerate(tgts):
                sp_t = psum.tile([128, TQ], F32, tag="C", bufs=2,
                                 name=f"sw{q}_{ti}")
                nc.tensor.matmul(sp_t, perm_sb, tgt, start=True, stop=True)
                sw_ps.append(sp_t)
            for jj in range(4):
                vtp = psum.tile([128, 128], F16, tag="C", bufs=2, name=f"vtp{q}_{jj}")
                nc.tensor.transpose(vtp, vt[:, 128 * jj:128 * (jj + 1)], ident)
                nc.scalar.copy(out=vS[q][:, jj, :], in_=vtp)
            for tgt, sp_t in zip(tgts, sw_ps):
                tmp = rpool.tile([128, TQ], F16, tag="tmp")
                nc.vector.tensor_mul(tmp, tgt, cs)
                swb = rpool.tile([128, TQ], F16, tag="swb")
                nc.vector.tensor_mul(swb, sp_t, sn)
                nc.vector.tensor_add(tgt, tmp, swb)

        def att_make(q):
            nj = 4 * (q + 1)
            L = nj // 2  # wide steps per head

            def blk(j):
                c0 = max(0, 128 * j - TQ * q)
                return c0, TQ - c0, j >= 4 * q

            def emit_swide(h, step):
                s_t = psum.tile([128, 2 * TQ], F32, tag="A", bufs=2,
                                name=f"s{q}_{h}_{step}")
                for u in (0, 1):
                    j = 2 * step + u
                    c0, N, diag = blk(j)
                    nc.tensor.matmul(
                        s_t[:, TQ * u:TQ * u + N],
                        kT[j // 4][:, 128 * (j % 4):128 * (j % 4) + 128],
                        qT[q][:, h, c0:TQ],
                        start=True, stop=not diag)
                    if diag:
                        nc.tensor.matmul(
                            s_t[:, TQ * u:TQ * u + 128], maskA, maskB,
                            start=False, stop=True)
                p_t = ppool.tile([128, 2 * TQ], F16, tag="p",
                                 name=f"p{q}_{h}_{step}")
                _, N0, _ = blk(2 * step)
                _, N1, _ = blk(2 * step + 1)
                if (q == 0 and step == 0) or N0 < TQ:
                    # split: shorter first-den wait / skip garbage columns
                    nc.scalar.activation(
                        p_t[:, 0:N0], s_t[:, 0:N0], AF.Exp, scale=SCALE)
                    nc.scalar.activation(
                        p_t[:, TQ:TQ + N1], s_t[:, TQ:TQ + N1], AF.Exp,
                        scale=SCALE)
                else:
                    nc.scalar.activation(
                        p_t[:, 0:TQ + N1], s_t[:, 0:TQ + N1], AF.Exp,
                        scale=SCALE)
                return p_t

            def emit_deno(h, step, oacc, dacc, p_t):
                for u in (0, 1):
                    j = 2 * step + u
                    c0, N, _ = blk(j)
                    st, sp = (j == 0), (j == nj - 1)
                    nc.tensor.matmul(dacc[:, c0:c0 + N], ones_sq,
                                     p_t[:, TQ * u:TQ * u + N],
                                     start=st, stop=sp)
                    nc.tensor.matmul(oacc[:, c0:c0 + N],
                                     vS[j // 4][:, j % 4, :],
                                     p_t[:, TQ * u:TQ * u + N],
                                     start=st, stop=sp)

            def normalize(h, oacc, dacc):
                inv = rpool.tile([128, TQ], F32, tag="inv")
                nc.vector.reciprocal_approx_fast(out=inv, in_=dacc)
                nc.vector.tensor_mul(oT[q][:, h, :], oacc, inv)

            # flattened pipeline over all heads: den/o lags s/exp so exps
            # overlap PE work; accumulators allocated at first den/o so the
            # PSUM rings stay ordered when the first s/exp groups are hoisted
            # before wo_proj(q-1).
            seq = [(h, step) for h in range(QH) for step in range(L)]
            state = {}  # h -> (oacc, dacc)
            pend = []   # [(h, step, p_t)]

            def pop_one():
                ph, pstep, pp = pend.pop(0)
                if ph not in state:
                    state[ph] = (
                        psum.tile([128, TQ], F32, tag="B", bufs=2,
                                  name=f"o{q}_{ph}"),
                        psum.tile([128, TQ], F32, tag="C", bufs=2,
                                  name=f"d{q}_{ph}"),
                    )
                emit_deno(ph, pstep, *state[ph], pp)
                if pstep == L - 1:
                    normalize(ph, *state[ph])

            def emit_pos(h, step, lag=1):
                pend.append((h, step, emit_swide(h, step)))
                if len(pend) > lag:
                    pop_one()

            def finish():
                while pend:
                    pop_one()

            return emit_pos, finish, seq

        def wo_proj(q):
            out_t = outsb.tile([128, CK, TQ], F16, tag="o", name=f"out{q}")
            for co in range(CK):
                op = psum.tile([128, TQ], F32, tag="B", bufs=2, name=f"op{q}_{co}")
                for h in range(QH):
                    nc.tensor.matmul(op, wo_sb[:, co, h, :], oT[q][:, h, :],
                                     start=(h == 0), stop=(h == QH - 1))
                if co % 2 == 0:
                    nc.scalar.copy(out=out_t[:, co, :], in_=op)
                else:
                    nc.vector.tensor_copy(out=out_t[:, co, :], in_=op)
                # stream output out as it completes; small final piece so the
                # end-of-kernel DMA drain is short
                ends = (3, 7, 11, 14, 15) if q == NQ - 1 else (3, 7, 11, 15)
                if co in ends:
                    lo = ([-1] + list(ends))[ends.index(co)] + 1
                    nc.gpsimd.dma_start(
                        out=outX[:, q, lo:co + 1, :],
                        in_=out_t[:, lo:co + 1, :])

        for q in range(NQ):
            proj(q)
            if q + 2 < NQ:
                x_load(q + 2)
            emit_pos, finish, seq = att_make(q)
            for hs in seq[:2]:  # hoist first s/exp groups over wo_proj
                emit_pos(*hs, lag=2)
            if q > 0:
                wo_proj(q - 1)
            for hs in seq[2:]:
                emit_pos(*hs)
            finish()
        wo_proj(NQ - 1)


_PERM = np.concatenate([np.arange(0, HD, 2), np.arange(1, HD, 2)])

PROFILE = False
LAST_EXEC_NS = None
LAST_RESULTS = None


def kernel(x, freqs_cos, freqs_sin, wq, wk, wv, wo):
    global LAST_EXEC_NS, LAST_RESULTS
    if "nc" not in _CACHE:
        _CACHE["nc"] = _build_nc()
    nc = _CACHE["nc"]

    x = np.asarray(x, dtype=np.float32)
    fc = np.asarray(freqs_cos, dtype=np.float32)
    fs = np.asarray(freqs_sin, dtype=np.float32)
    wq = np.asarray(wq, dtype=np.float32)
    wk = np.asarray(wk, dtype=np.float32)
    wv = np.asarray(wv, dtype=np.float32)
    wo = np.asarray(wo, dtype=np.float32)

    cosT = fc.T                                   # [64, T]
    sinT = fs.T
    cosX = np.ascontiguousarray(
        np.concatenate([cosT, cosT], axis=0)).astype(np.float16)   # [128, T]
    sinX = np.ascontiguousarray(
        np.concatenate([-sinT, sinT], axis=0)).astype(np.float16)
    # half-swap permutation: (P.T @ q)[s] = q[s ^ 64]
    permX = np.zeros((128, 128), dtype=np.float16)
    permX[np.arange(128), np.arange(128) ^ 64] = 1.0

    in_maps = []
    for core in range(8):
        b, g = core // 4, core % 4
        # x: [p, q, k, t] = x[b, 512q+t, 128k+p]
        xb = x[b].astype(np.float16)              # [T, C]
        xQd = np.ascontiguousarray(
            xb.reshape(NQ, TQ, CK, 128).transpose(3, 0, 2, 1))
        # wq (rope-permuted rows): wqX[p, k, d] = wq_g.T[128k+p, d]
        wq_g = wq[512 * g:512 * (g + 1)].reshape(QH, HD, C)[:, _PERM, :]
        wqT = wq_g.reshape(QH * HD, C).T.astype(np.float16)        # [C, 512]
        wqX = np.ascontiguousarray(
            wqT.reshape(CK, 128, QH * HD).transpose(1, 0, 2))
        wkT = wk[HD * g:HD * (g + 1)][_PERM].T.astype(np.float16)  # [C, 128]
        wkX = np.ascontiguousarray(wkT.reshape(CK, 128, HD).transpose(1, 0, 2))
        wvT = wv[HD * g:HD * (g + 1)].T.astype(np.float16)
        wvX = np.ascontiguousarray(wvT.reshape(CK, 128, HD).transpose(1, 0, 2))
        # wo: woX[p, co, h, d] = wo[128co+d, 512g+128h+p]
        wo_g = wo[:, 512 * g:512 * (g + 1)].astype(np.float16)     # [C, 512]
        woX = np.ascontiguousarray(
            wo_g.reshape(CK, 128, QH, 128).transpose(3, 0, 2, 1))
        in_maps.append({
            "xQ": xQd, "wqX": wqX, "wkX": wkX, "wvX": wvX, "woX": woX,
            "cosX": cosX, "sinX": sinX, "permX": permX,
        })

    res = run_bass_kernel_spmd(nc, in_maps, list(range(8)), trace=PROFILE)
    LAST_EXEC_NS = res.exec_time_ns
    LAST_RESULTS = res

    out = np.empty((B, T, C), dtype=np.float32)
    for b in range(B):
        acc = res.results[4 * b]["outX"].astype(np.float32)
        for g in range(1, 4):
            acc = acc + res.results[4 * b + g]["outX"]
        # outX[d, q, co, t] -> out[512q+t, 128co+d]
        out[b] = acc.transpose(1, 3, 2, 0).reshape(T, C)
    return out
